# revision 33
# baseline (speedup 1.0000x reference)
"""MemN2N Bass kernel builder (per-core program, SPMD over 8 cores).

Per-core work (core c):
  - 8 local batches (B_LOC). story_pad [TOT_SLOTS, SENT] int32 staged so that
    slot(16b+q, j) = batch b, sentence 13q+j  (S_PAD sents/batch, SPP per part).
    Pad tokens point at table row V (a zero row appended host-side).
  - embC [V+1, 4E] bf16: the 4 embedding tables concatenated column-wise.
    Story gather: per slot-group j (SPP of them), GK CCE-accumulating indirect
    DMAs into gbuf [128, GSL, 4E] bf16, then one DVE reduce -> Gall[:, j, :].
    One 1KB descriptor per token covers all 4 tables.
  - Question: tokens spread over all 128 partitions ([128, QSL] idx), one
    indirect gather + reduce -> qsum, then matmul with qmask -> uT [E, B_LOC].
  - 3 attention hops fully on-chip (bf16 matmuls): scores = uT.T @ GT[h] with
    slot layout col c = 128*j + p; additive mask (host-staged) makes softmax
    over the full row equal the per-batch softmax; probs repacked to slot
    partitions via SPP PE transposes; combine matmuls accumulate uc.
  - logits = u3 @ emb3.T via emb3T bf16 [E, VPAD] staged pre-transposed;
    vocab softmax on-device; exp values transposed per-batch with a
    rec-scaled identity (normalization folded into the PE transpose) and
    DMA'd straight from PSUM. Output [B_LOC, VPAD] f32, host slices to V.
"""
import sys

sys.path.insert(0, "/opt/trn_rl_repo")

from contextlib import ExitStack

import numpy as np

import concourse.bass as bass
import concourse.mybir as mybir
import concourse.tile as tile
from concourse.masks import make_identity

F32 = mybir.dt.float32
BF16 = mybir.dt.bfloat16
I32 = mybir.dt.int32
AX = mybir.AxisListType
ALU = mybir.AluOpType
ACTF = mybir.ActivationFunctionType

P = 128
E = 128


class Cfg:
    def __init__(self, B_LOC=8, S=200, SENT=50, V=100000, K_HOP=3, CHUNK_VT=32):
        self.B_LOC = B_LOC
        self.S = S
        self.SENT = SENT
        self.V = V
        self.K_HOP = K_HOP
        self.NT = K_HOP + 1
        self.CE = self.NT * E  # concat table width
        self.PPB = P // B_LOC  # partitions per batch
        self.SPP = -(-(B_LOC * S) // P)  # ceil: slot groups per partition
        self.S_PAD = self.PPB * self.SPP
        assert self.S_PAD >= S
        self.TOT_SLOTS = P * self.SPP
        # CCE accumulate slots for the story gather (token s -> slot s%NSLOT)
        self.NSLOT = 5
        # question: tokens spread over 128 partitions
        self.QSL = -(-SENT // self.PPB)
        # vocab tiling
        self.NVT = -(-V // P)
        self.VPAD = self.NVT * P
        self.LAST_VT_ROWS = V - (self.NVT - 1) * P
        self.CHUNK_VT = CHUNK_VT
        self.NCH = -(-self.NVT // CHUNK_VT)
        # final-phase per-batch transpose groups (128 V-tiles each)
        self.NG = -(-self.NVT // P)  # 7
        self.NG_FULL = self.NVT // P  # 6
        self.LAST_G_VT = self.NVT - self.NG_FULL * P  # 14


def build_kernel(cfg: Cfg, nc: bass.Bass):
    c = cfg
    story = nc.declare_dram_parameter("story_pad", [c.TOT_SLOTS, c.SENT], I32, isOutput=False)
    qidx = nc.declare_dram_parameter("qidx", [P, c.QSL], I32, isOutput=False)
    embC = nc.declare_dram_parameter("embC", [c.V + 1, c.CE], BF16, isOutput=False)
    emb3T = nc.declare_dram_parameter("emb3T", [E, c.VPAD], BF16, isOutput=False)
    qmask = nc.declare_dram_parameter("qmask", [P, c.B_LOC], F32, isOutput=False)
    maskT = nc.declare_dram_parameter("maskT", [c.B_LOC, c.TOT_SLOTS], BF16, isOutput=False)
    out = nc.declare_dram_parameter("out", [c.B_LOC, c.VPAD], F32, isOutput=True)

    with tile.TileContext(nc) as tc:
        _body(c, nc, tc, story, qidx, embC, emb3T, qmask, maskT, out)
    return nc


def _body(c: Cfg, nc, tc, story, qidx, embC, emb3T, qmask, maskT, out):
    with ExitStack() as es:
        # ---------- persistent SBUF pools (no reuse: maximizes DMA overlap) ----
        cpool = es.enter_context(tc.tile_pool(name="const", bufs=1))
        gpool = es.enter_context(tc.tile_pool(name="G", bufs=1))
        upool = es.enter_context(tc.tile_pool(name="u", bufs=1))
        qpool = es.enter_context(tc.tile_pool(name="qg", bufs=1))
        gbpool = es.enter_context(tc.tile_pool(name="gather", bufs=2))
        hpool = es.enter_context(tc.tile_pool(name="hop", bufs=1))
        fpool = es.enter_context(tc.tile_pool(name="fin", bufs=1))
        epool = es.enter_context(tc.tile_pool(name="emb3c", bufs=16))
        osb = es.enter_context(tc.tile_pool(name="outsb", bufs=3))

        identity = cpool.tile([P, P], F32)
        make_identity(nc, identity[:])
        identity_bf = cpool.tile([P, P], BF16)
        nc.vector.tensor_copy(out=identity_bf[:], in_=identity[:])

        idx_t = cpool.tile([P, c.SPP * c.SENT], I32)
        nc.sync.dma_start(
            out=idx_t[:], in_=story[:].rearrange("(p j) t -> p (j t)", p=P)
        )
        qidx_t = cpool.tile([P, c.QSL], I32)
        nc.sync.dma_start(out=qidx_t[:], in_=qidx[:])
        qmask_t = cpool.tile([P, c.B_LOC], F32)
        nc.sync.dma_start(out=qmask_t[:], in_=qmask[:])
        maskT_t = cpool.tile([c.B_LOC, c.TOT_SLOTS], BF16)
        nc.sync.dma_start(out=maskT_t[:], in_=maskT[:])

        Gall = gpool.tile([P, c.SPP, c.CE], BF16, name="Gall")
        GT = [gpool.tile([P, c.TOT_SLOTS], BF16, tag=f"GT{t}", name=f"GT{t}") for t in range(c.K_HOP)]

        # ---------- question gather -> uT ----------
        with tc.tile_pool(name="q_ps", bufs=1, space="PSUM") as qps:
            qb = qpool.tile([P, c.QSL, c.CE], BF16)
            for g in range(c.QSL):
                nc.gpsimd.indirect_dma_start(
                    out=qb[:, g, :],
                    out_offset=None,
                    in_=embC[:],
                    in_offset=bass.IndirectOffsetOnAxis(
                        ap=qidx_t[:, g : g + 1], axis=0
                    ),
                )
            qsum = qpool.tile([P, E], F32)
            nc.vector.tensor_reduce(
                out=qsum[:].unsqueeze(-1),
                in_=qb[:, :, :E].rearrange("p g e -> p e g"),
                axis=AX.X,
                op=ALU.add,
            )
            uT = upool.tile([P, c.B_LOC], F32, tag="uT0")
            tpu = qps.tile([P, c.B_LOC], F32)
            nc.tensor.matmul(
                out=tpu[:], lhsT=qsum[:], rhs=qmask_t[:], start=True, stop=True
            )
            nc.vector.tensor_copy(out=uT[:], in_=tpu[:])

        # ---------- story gather + CCE-accumulate + reduce + fused transposes ----
        # hop-0 score matmuls run inside the gather loop (per transposed block),
        # so the hop psum pools wrap both phases. The additive softmax mask is
        # pre-accumulated into the score psum by a tiny identity matmul, so the
        # exp reads psum directly and the DVE queue stays pure reduces.
        with (
            tc.tile_pool(name="tp", bufs=2, space="PSUM") as tppool,
            tc.tile_pool(name="hop_sc", bufs=1, space="PSUM") as hsc,
            tc.tile_pool(name="hop_pt", bufs=1, space="PSUM") as hpt,
            tc.tile_pool(name="hop_uc", bufs=1, space="PSUM") as huc,
            nc.allow_low_precision(reason="bf16 G accumulate, rel tol 2e-2"),
        ):
            uT_bf0 = hpool.tile([P, c.B_LOC], BF16, tag="uTbf0")
            nc.vector.tensor_copy(out=uT_bf0[:], in_=uT[:])
            sc_ps0 = hsc.tile([c.B_LOC, c.TOT_SLOTS], F32, tag="sc")

            # HW indirect DMA consumes ONE index per partition per
            # instruction, so the gather is one instr per (slot-group, token).
            # Token s CCE-accumulates into gbuf slot s % NSLOT: the 5-apart
            # WAW chain hides the DMA-complete sem under 4 other gathers.
            # Reduces are split per table (ra: t0, rb1: t1, rb23: t2+t3) so
            # hop 0 unblocks right after the last gather.
            gbufs = {}

            def emit_j(j):
                for s in range(c.SENT):
                    nc.gpsimd.indirect_dma_start(
                        out=gbufs[j][:, s % c.NSLOT, :],
                        out_offset=None,
                        in_=embC[:],
                        in_offset=bass.IndirectOffsetOnAxis(
                            ap=idx_t[:, j * c.SENT + s : j * c.SENT + s + 1],
                            axis=0,
                        ),
                        compute_op=(ALU.bypass if s < c.NSLOT else ALU.add),
                    )

            def red(j, e0, e1):
                nc.vector.tensor_reduce(
                    out=Gall[:, j, e0:e1].unsqueeze(-1),
                    in_=gbufs[j][:, :, e0:e1].rearrange("p g e -> p e g"),
                    axis=AX.X,
                    op=ALU.add,
                )

            def transpose_t(t, j):
                tp = tppool.tile([P, P], F32, tag="tp")
                nc.tensor.matmul(
                    out=tp[:],
                    lhsT=Gall[:, j, t * E : (t + 1) * E],
                    rhs=identity_bf[:],
                    start=True,
                    stop=True,
                )
                nc.scalar.activation(
                    out=GT[t][:, j * P : (j + 1) * P], in_=tp[:], func=ACTF.Copy
                )

            def mask_mm(sc_ps, c0, c1):
                nc.tensor.matmul(
                    out=sc_ps[:, c0:c1],
                    lhsT=identity_bf[: c.B_LOC, : c.B_LOC],
                    rhs=maskT_t[:, c0:c1],
                    start=True,
                    stop=False,
                    skip_group_check=True,
                )

            def score_mm(sc_ps, uTb, h, c0, c1):
                nc.tensor.matmul(
                    out=sc_ps[:, c0:c1],
                    lhsT=uTb[:],
                    rhs=GT[h][:, c0:c1],
                    start=False,
                    stop=True,
                    skip_group_check=True,
                )

            for j in range(c.SPP):
                gbufs[j] = gbpool.tile([P, c.NSLOT, c.CE], BF16, tag="gbuf", name=f"gbuf{j}")
                emit_j(j)
                red(j, 0, E)
                if j >= 1:
                    red(j - 1, E, 2 * E)
                if j >= 2:
                    red(j - 2, 2 * E, c.CE)
                transpose_t(0, j)
                mask_mm(sc_ps0, j * P, (j + 1) * P)
                score_mm(sc_ps0, uT_bf0, 0, j * P, (j + 1) * P)
                if j >= 1:
                    transpose_t(1, j - 1)
                if j >= 2:
                    transpose_t(2, j - 2)
            J = c.SPP - 1
            red(J, E, 2 * E)
            transpose_t(1, J)

            # ---------- K_HOP attention hops (fully on-chip, bf16 matmuls) ---
            def hop_front(h, sc_ps):
                probs = hpool.tile([c.B_LOC, c.TOT_SLOTS], F32, tag="probs", name=f"probs{h}")
                denom = hpool.tile([c.B_LOC, 1], F32, tag="denom", name=f"denom{h}")
                nc.scalar.activation(
                    out=probs[:], in_=sc_ps[:], func=ACTF.Exp, accum_out=denom[:]
                )
                rec = hpool.tile([c.B_LOC, 1], F32, tag="rec", name=f"rec{h}")
                nc.vector.reciprocal(out=rec[:], in_=denom[:])
                # diag(rec): folds the softmax normalization into the repack
                idr = hpool.tile([c.B_LOC, c.B_LOC], F32, tag="idr", name=f"idr{h}")
                nc.vector.tensor_scalar_mul(
                    idr[:], identity[: c.B_LOC, : c.B_LOC], rec[:]
                )
                # repack probs to slot partitions: SPP transposes [8,128] -> [128,8]
                ptp = hpt.tile([P, c.SPP * c.B_LOC], F32, tag="ptp", name=f"ptp{h}")
                for m in range(c.SPP):
                    nc.tensor.matmul(
                        out=ptp[:, m * c.B_LOC : (m + 1) * c.B_LOC],
                        lhsT=probs[:, m * P : (m + 1) * P],
                        rhs=idr[:],
                        start=True,
                        stop=True,
                    )
                pslot = hpool.tile([P, c.SPP * c.B_LOC], BF16, tag="pslot", name=f"pslot{h}")
                nc.scalar.activation(out=pslot[:], in_=ptp[:], func=ACTF.Copy)
                return pslot

            def hop_combine(h, pslot, uT):
                uc_ps = huc.tile([P, c.B_LOC], F32, tag="uc", name=f"uc{h}")
                for m in range(c.SPP):
                    nc.tensor.matmul(
                        out=uc_ps[:],
                        lhsT=Gall[:, m, (h + 1) * E : (h + 2) * E],
                        rhs=pslot[:, m * c.B_LOC : (m + 1) * c.B_LOC],
                        start=(m == 0),
                        stop=(m == c.SPP - 1),
                    )
                uT_new = upool.tile([P, c.B_LOC], F32, tag=f"uT{h + 1}")
                nc.vector.tensor_add(out=uT_new[:], in0=uc_ps[:], in1=uT[:])
                return uT_new

            # hop 0: scores already accumulated; rb23 of the last two groups
            # runs under the hop-0 front so DVE never blocks the exp chain
            pslot0 = hop_front(0, sc_ps0)
            red(J - 1, 2 * E, c.CE)
            red(J, 2 * E, c.CE)
            transpose_t(2, J - 1)
            transpose_t(2, J)
            uT = hop_combine(0, pslot0, uT)

            for h in range(1, c.K_HOP):
                uT_bf = hpool.tile([P, c.B_LOC], BF16, tag=f"uTbf{h}")
                nc.vector.tensor_copy(out=uT_bf[:], in_=uT[:])
                sc_ps = hsc.tile([c.B_LOC, c.TOT_SLOTS], F32, tag="sc", name=f"sc{h}")
                for c0 in range(0, c.TOT_SLOTS, 512):
                    c1 = min(c0 + 512, c.TOT_SLOTS)
                    mask_mm(sc_ps, c0, c1)
                    score_mm(sc_ps, uT_bf, h, c0, c1)
                pslot = hop_front(h, sc_ps)
                uT = hop_combine(h, pslot, uT)

        # ---------- final phase
        # ---------- final phase: logits + vocab softmax ----------
        with (
            tc.tile_pool(name="fin_ps", bufs=2, space="PSUM") as fps,
            tc.tile_pool(name="den_ps", bufs=1, space="PSUM") as dps,
            tc.tile_pool(name="out_ps", bufs=2, space="PSUM") as ops,
        ):
            uT_bf = fpool.tile([P, c.B_LOC], BF16)
            nc.vector.tensor_copy(out=uT_bf[:], in_=uT[:])
            ones = fpool.tile([P, P], BF16)
            nc.vector.memset(ones[:], 1.0)
            ones_part = fpool.tile([P, P], BF16)
            nc.vector.memset(ones_part[:], 0.0)
            nc.vector.memset(ones_part[: c.LAST_VT_ROWS, :], 1.0)

            exp_buf = fpool.tile([P, c.NVT * c.B_LOC], BF16)
            CW = c.CHUNK_VT * c.B_LOC
            den_ps = dps.tile([P, CW], F32)
            for ch in range(c.NCH):
                vt0 = ch * c.CHUNK_VT
                nvt = min(c.CHUNK_VT, c.NVT - vt0)
                echunk = epool.tile([P, c.CHUNK_VT * P], BF16, tag="echunk")
                # prefetched chunks stream on SP under the gather; tail chunks
                # alternate SP/Pool so ACT stays free for the exp chain
                if ch < 16:
                    eng = nc.sync
                else:
                    eng = (nc.sync, nc.gpsimd)[ch % 2]
                eng.dma_start(
                    out=echunk[:, : nvt * P],
                    in_=emb3T[:, vt0 * P : (vt0 + nvt) * P],
                )
                lg_ps = fps.tile([P, CW], F32, tag="lg")
                for m in range(nvt):
                    nc.tensor.matmul(
                        out=lg_ps[:, m * c.B_LOC : (m + 1) * c.B_LOC],
                        lhsT=echunk[:, m * P : (m + 1) * P],
                        rhs=uT_bf[:],
                        start=True,
                        stop=True,
                    )
                ecols = nvt * c.B_LOC
                nc.scalar.activation(
                    out=exp_buf[:, vt0 * c.B_LOC : vt0 * c.B_LOC + ecols],
                    in_=lg_ps[:, :ecols],
                    func=ACTF.Exp,
                )
                exp_ch = exp_buf[:, vt0 * c.B_LOC : vt0 * c.B_LOC + ecols]
                last_has_partial = vt0 + nvt == c.NVT and c.LAST_VT_ROWS < P
                full_cols = ecols - (c.B_LOC if last_has_partial else 0)
                if full_cols > 0:
                    nc.tensor.matmul(
                        out=den_ps[:, :full_cols],
                        lhsT=ones[:],
                        rhs=exp_ch[:, :full_cols],
                        start=(ch == 0),
                        stop=False,
                        skip_group_check=True,
                    )
                if last_has_partial:
                    nc.tensor.matmul(
                        out=den_ps[:, full_cols:ecols],
                        lhsT=ones_part[:],
                        rhs=exp_ch[:, full_cols:ecols],
                        start=False,
                        stop=True,
                        skip_group_check=True,
                    )
            # denominators: every row of den_ps already holds sum_p exp[p, col]
            den8 = fpool.tile([P, c.B_LOC], F32)
            nc.vector.tensor_reduce(
                out=den8[:].unsqueeze(-1),
                in_=den_ps[:].rearrange("o (m b) -> o b m", b=c.B_LOC),
                axis=AX.X,
                op=ALU.add,
            )
            rec8 = fpool.tile([P, c.B_LOC], F32)
            nc.vector.reciprocal(out=rec8[:], in_=den8[:])
            rec8_bf = fpool.tile([P, c.B_LOC], BF16)
            nc.vector.tensor_copy(out=rec8_bf[:], in_=rec8[:])
            # rec-scaled identities: fold normalization into the PE transposes
            idsc = fpool.tile([P, c.B_LOC, P], BF16)
            nc.vector.tensor_tensor(
                out=idsc[:],
                in0=identity_bf[:].unsqueeze(1).to_broadcast([P, c.B_LOC, P]),
                in1=rec8_bf[:].unsqueeze(-1).to_broadcast([P, c.B_LOC, P]),
                op=ALU.mult,
            )

            # per-batch transposes of exp columns; copy to SBUF then batched DMA
            expv = exp_buf[:].rearrange("p (t b) -> p b t", b=c.B_LOC)
            for b in range(c.B_LOC):
                otp = ops.tile([P, c.NG * P], F32, tag="otp")
                for g in range(c.NG):
                    vt0 = g * P
                    nvt = min(P, c.NVT - vt0)
                    nc.tensor.matmul(
                        out=otp[:nvt, g * P : (g + 1) * P],
                        lhsT=expv[:, b, vt0 : vt0 + nvt],
                        rhs=idsc[:, b, :],
                        start=True,
                        stop=True,
                    )
                osbuf = osb.tile([P, c.NG * P], F32, tag="osb")
                if b % 2 == 0:
                    nc.vector.tensor_copy(
                        out=osbuf[:, : c.NG_FULL * P], in_=otp[:, : c.NG_FULL * P]
                    )
                    nc.vector.tensor_copy(
                        out=osbuf[: c.LAST_G_VT, c.NG_FULL * P :],
                        in_=otp[: c.LAST_G_VT, c.NG_FULL * P :],
                    )
                else:
                    nc.scalar.activation(
                        out=osbuf[:, : c.NG_FULL * P],
                        in_=otp[:, : c.NG_FULL * P],
                        func=ACTF.Copy,
                    )
                    nc.scalar.activation(
                        out=osbuf[: c.LAST_G_VT, c.NG_FULL * P :],
                        in_=otp[: c.LAST_G_VT, c.NG_FULL * P :],
                        func=ACTF.Copy,
                    )
                eng = nc.sync if b % 2 == 0 else nc.gpsimd
                eng.dma_start(
                    out=out[b : b + 1, : c.NG_FULL * P * P].rearrange(
                        "one (g t col) -> one t g col", g=c.NG_FULL, col=P
                    ),
                    in_=osbuf[:, : c.NG_FULL * P].rearrange("p (g col) -> p g col", col=P),
                )
                eng.dma_start(
                    out=out[b : b + 1, c.NG_FULL * P * P :].rearrange(
                        "one (t col) -> one t col", col=P
                    ),
                    in_=osbuf[: c.LAST_G_VT, c.NG_FULL * P :],
                )


# ---------------- host-side pack/unpack ----------------
def _shared_arrays(c: Cfg, emb_A: np.ndarray):
    import ml_dtypes

    embC = np.zeros((c.V + 1, c.CE), dtype=ml_dtypes.bfloat16)
    for t in range(c.NT):
        embC[: c.V, t * E : (t + 1) * E] = emb_A[t].astype(ml_dtypes.bfloat16)
    e3T = np.zeros((E, c.VPAD), np.float32)
    e3T[:, : c.V] = emb_A[c.NT - 1].T
    qmask = np.zeros((P, c.B_LOC), np.float32)
    for b in range(c.B_LOC):
        qmask[b * c.PPB : (b + 1) * c.PPB, b] = 1.0
    # maskT[b, c] = 0 where col c = 128*j + p is a real sentence of batch b
    cc = np.arange(c.TOT_SLOTS)
    j = cc // P
    p = cc % P
    b_of = p // c.PPB
    s = c.SPP * (p % c.PPB) + j
    maskT = np.full((c.B_LOC, c.TOT_SLOTS), -50.0, np.float32)
    for b in range(c.B_LOC):
        maskT[b, (b_of == b) & (s < c.S)] = 0.0
    maskT = maskT.astype(ml_dtypes.bfloat16)
    return {
        "embC": embC,
        "emb3T": e3T.astype(ml_dtypes.bfloat16),
        "qmask": qmask,
        "maskT": maskT,
    }


def _pack_story(c: Cfg, story_c: np.ndarray):
    story_pad = np.full((c.B_LOC, c.S_PAD, c.SENT), c.V, np.int32)
    story_pad[:, : c.S, :] = story_c
    return np.ascontiguousarray(story_pad.reshape(c.TOT_SLOTS, c.SENT))


def _pack_quest(c: Cfg, quest_c: np.ndarray):
    qpad = np.full((c.B_LOC, c.QSL * c.PPB), c.V, np.int32)
    qpad[:, : c.SENT] = quest_c
    return np.ascontiguousarray(
        qpad.reshape(c.B_LOC, c.QSL, c.PPB).transpose(0, 2, 1).reshape(P, c.QSL)
    )


def pack_core_inputs(cfg: Cfg, story_c: np.ndarray, quest_c: np.ndarray, emb_A: np.ndarray):
    return {
        "story_pad": _pack_story(cfg, story_c),
        "qidx": _pack_quest(cfg, quest_c),
        **_shared_arrays(cfg, emb_A),
    }


def ref_numpy(story, question, emb_A):
    """Full-batch numpy reference (mirrors reference.py)."""
    K_HOP = emb_A.shape[0] - 1
    u = emb_A[0][question].sum(axis=1)
    for i in range(K_HOP):
        m = emb_A[i][story].sum(axis=2)
        cc = emb_A[i + 1][story].sum(axis=2)
        logits_att = np.einsum("bse,be->bs", m, u)
        pa = np.exp(logits_att - logits_att.max(-1, keepdims=True))
        probs = pa / pa.sum(-1, keepdims=True)
        u = np.einsum("bse,bs->be", cc, probs) + u
    logits = u @ emb_A[-1].T
    z = np.exp(logits - logits.max(-1, keepdims=True))
    return (z / z.sum(-1, keepdims=True)).astype(np.float32)


N_CORES = 8
_CACHE = {}


def _get_nc(cfg):
    key = ("nc", cfg.B_LOC, cfg.S, cfg.SENT, cfg.V, cfg.K_HOP)
    if key not in _CACHE:
        import concourse.bacc as bacc

        nc = bacc.Bacc(target_bir_lowering=False)
        build_kernel(cfg, nc)
        nc.finalize()
        _CACHE[key] = nc
    return _CACHE[key]


def _pack_shared(cfg, emb_A):
    key = "shared"
    if key not in _CACHE or _CACHE[key][0] is not emb_A:
        _CACHE[key] = (emb_A, _shared_arrays(cfg, emb_A))
    return _CACHE[key][1]


def kernel(story, question, emb_A, _trace=False, _trace_kwargs=None):
    from concourse import bass_utils

    story = np.asarray(story)
    question = np.asarray(question)
    emb_A = np.asarray(emb_A, dtype=np.float32)

    cfg = Cfg(
        B_LOC=story.shape[0] // N_CORES,
        S=story.shape[1],
        SENT=story.shape[2],
        V=emb_A.shape[1],
        K_HOP=emb_A.shape[0] - 1,
    )
    nc = _get_nc(cfg)
    shared = _pack_shared(cfg, emb_A)
    in_maps = []
    for ci in range(N_CORES):
        sl = slice(ci * cfg.B_LOC, (ci + 1) * cfg.B_LOC)
        in_maps.append(
            {
                "story_pad": _pack_story(cfg, story[sl]),
                "qidx": _pack_quest(cfg, question[sl]),
                **shared,
            }
        )
    kwargs = {}
    if _trace:
        kwargs = dict(trace=True, trace_kwargs=_trace_kwargs or {})
    res = bass_utils.run_bass_kernel_spmd(
        nc, in_maps, core_ids=list(range(N_CORES)), **kwargs
    )
    out = np.concatenate([r["out"][:, : cfg.V] for r in res.results], axis=0)
    if _trace:
        return out, res
    return out
